# revision 76
# baseline (speedup 1.0000x reference)
"""Trainium2 Bass kernel for nn_BondDecoder (topk_masking).

Strategy (v3 — engine-balanced, software-pipelined, Pool-free hot loop):
  - Data-parallel over batch: 64 batches -> 8 cores x 8 slots, compacted to
    unmasked positions per batch (sorted + dealt so SPMD slot sizes match).
  - k-side projection biases are DROPPED (softmax is invariant to per-row
    score shifts), so virtual pad columns (x=0) give E_pad = exp(0) = 1
    exactly and the denominator fix is Sc = S_raw - npad ([il,1] data).
    q-side biases ride as rank-1 matmuls in the projection PSUM groups, so
    the PSUM->SBUF q/k copies are bias-free, 2 banks per op (ACT/DVE split).
  - Per chunk (il<=128 rows, w cols): 8 head-score matmuls into 2-bank PSUM
    ring tiles; 4 exps (PSUM->SBUF f16); 8 DVE 4x-mode row-sums
    (tensor_scalar + accum_out); signed Sc / reciprocal; 8 DVE 4x prescales
    P_h = E_h * (+-1/Sc_h); PE sums the 8 maps + the host A-map
    (A = C_src - C_tgt*(1-t_i)(1-t_j)) via identity-weight matmuls
    accumulated in PSUM (f32); ACT squares; one fused DVE
    scalar_tensor_tensor computes msq*g with accum_out = the row loss
    (g = host-packed (1 - t_i t_j) * valid masks, so no or-mask/pad fixups).
  - Software pipelining: each chunk's combine (PE idMMs + ACT square) and
    final reduce (DVE) are emitted PIPE_DEPTH chunks later, so late-dep ops
    never head-of-line block the next chunks' early work on in-order engines.
  - GpSimd/Pool runs NOTHING in the hot loop (its SBUF port arbitrates with
    DVE 2-port perf modes on HW); it only issues the 3 startup weight DMAs.
    All data DMAs are prefetched up-front on the SP HWDGE queue.
  - Leftover rows (256..n) of big slots are consolidated into one shared
    final chunk (32-aligned lanes; k-pad columns of big slots are zeroed so
    lane matmuls span full wmax and pads reduce to the npad correction).
  - Final: ones-vector matmul over partitions + 3-column fold per slot.
"""

import sys
from contextlib import ExitStack

if "/opt/trn_rl_repo" not in sys.path:
    sys.path.insert(0, "/opt/trn_rl_repo")

import numpy as np

import concourse.bacc as bacc
import concourse.tile as tile
from concourse import bass_utils, mybir

L, B, DIM = 512, 64, 256
H, HD, MB = 4, 64, 6
NCORES = 8
BPC = B // NCORES  # slots per core

F32 = mybir.dt.float32
F16 = mybir.dt.float16
EDT = mybir.dt.float16
NP_EDT = np.float16

_CACHE = {}


def _cons_split(slot_n):
    """Slots whose rows 256..n go to the consolidated chunk. PE matmul
    output base partitions must be 32-aligned ({0,32,64}), so each slot
    gets a 32-wide lane (up to 3 slots, leftover count <= 32 each)."""
    bigs, offs = [], {}
    for s, n in enumerate(slot_n):
        c = n - 256
        if 0 < c <= 32 and len(bigs) < 3:
            offs[s] = 32 * len(bigs)
            bigs.append(s)
    tot = 0
    if bigs:
        last = bigs[-1]
        tot = offs[last] + (slot_n[last] - 256)
    return bigs, offs, tot


def _build_program(slot_n, nmax, totc):
    nc = bacc.Bacc(
        "TRN2",
        target_bir_lowering=False,
        debug=False,
        enable_asserts=False,
        num_devices=NCORES,
    )
    AL = mybir.AluOpType
    AF = mybir.ActivationFunctionType

    bigs, cons_off, ptot = _cons_split(slot_n)
    wmax = nmax          # slot width == rounded row count (no extra pad cols)
    wpad = wmax + 8      # amap row pitch: +8 cols for npad scalars

    xT_d = nc.dram_tensor("xT", [DIM, totc], F16, kind="ExternalInput").ap()
    wall_d = nc.dram_tensor("wall", [4, DIM, DIM], F16, kind="ExternalInput").ap()
    wqb_d = nc.dram_tensor("wqb", [1, 4, 128], F16, kind="ExternalInput").ap()
    eye_d = nc.dram_tensor("eye", [128, 128], F16, kind="ExternalInput").ap()
    amap_d = nc.dram_tensor(
        "amap", [BPC + 1, 7 * 128, wpad], F16, kind="ExternalInput"
    ).ap()
    loss_d = nc.dram_tensor("loss", [1, BPC], F32, kind="ExternalOutput").ap()

    with ExitStack() as ctx:
        tc = ctx.enter_context(tile.TileContext(nc))
        singles = ctx.enter_context(tc.tile_pool(name="singles", bufs=1))
        xapool = ctx.enter_context(tc.tile_pool(name="xapool", bufs=BPC + 1))
        xpool = ctx.enter_context(tc.tile_pool(name="xpool", bufs=BPC + 1))
        qk = ctx.enter_context(tc.tile_pool(name="qk", bufs=BPC + 1))
        epool = ctx.enter_context(tc.tile_pool(name="epool", bufs=7))
        work = ctx.enter_context(tc.tile_pool(name="work", bufs=10))
        small = ctx.enter_context(tc.tile_pool(name="small", bufs=12))
        pscp = ctx.enter_context(tc.tile_pool(name="psc", bufs=4, space="PSUM"))

        # ---- weights / constants (once per core) ----
        wqbt = singles.tile([1, 4, 128], F16, tag="wqbt")
        nc.gpsimd.dma_start(out=wqbt, in_=wqb_d)
        wt = singles.tile([128, 8, DIM], F16, tag="wt")
        nc.scalar.dma_start(
            out=wt, in_=wall_d.rearrange("a (g p) d -> p (a g) d", p=128)
        )
        w0 = [wt[:, 2 * p, :] for p in range(4)]
        w1 = [wt[:, 2 * p + 1, :] for p in range(4)]
        eye = singles.tile([128, 128], F16, tag="eye")
        nc.scalar.dma_start(out=eye, in_=eye_d)

        ones128 = singles.tile([128, 1], F32, tag="ones128")
        nc.vector.memset(ones128, 1.0)
        warm = singles.tile([1, 8], F32, tag="warm")
        nc.vector.memset(warm, 0.0)
        nc.scalar.activation(out=warm, in_=warm, func=AF.Exp)
        onesw = singles.tile([1, wpad], F16, tag="onesw")
        nc.vector.memset(onesw, 1.0)
        res = singles.tile([128, BPC * 3], F32, tag="res")
        nc.vector.memset(res, 0.0)
        consc = singles.tile([128, 1], F32, tag="consc")

        # per-slot persistent handles (for the consolidated tail chunk)
        S_qkt = {}
        holder = {}

        def load_slot(s, n, off):
            w = n
            xall = xapool.tile([128, 2, wmax], F16, tag="xall")
            nc.sync.dma_start(
                out=xall[:, :, 0:w],
                in_=xT_d[0:256, off : off + w].rearrange("(a p) c -> p a c", p=128),
            )
            amtile = xpool.tile([128, 7, wpad], F16, tag="am", name=f"am{s}")
            nc.sync.dma_start(
                out=amtile,
                in_=amap_d[s].rearrange("(c p) w -> p c w", p=128),
            )
            npf = xpool.tile([128, 2], F32, tag="npf", name=f"npf{s}")
            nc.vector.tensor_copy(out=npf, in_=amtile[:, 6, wmax : wmax + 2])
            return xall, amtile, npf

        def project(s, n, xall):
            w = n
            qkt = {}
            for br in range(2):
                for g in range(2):
                    pp = pscp.tile([128, 2, 512], F32, tag="psc")
                    for j, p in enumerate((2 * br, 2 * br + 1)):
                        mg = slice(128 * g, 128 * g + 128)
                        if j == 0:
                            # q side: rank-1 bias matmul opens the group
                            nc.tensor.matmul(
                                pp[:, j, 0:w],
                                wqbt[0:1, 2 * br + g, 0:128],
                                onesw[0:1, 0:w],
                                start=True, stop=False,
                            )
                            st = False
                        else:
                            st = True
                        nc.tensor.matmul(
                            pp[:, j, 0:w], w0[p][:, mg], xall[:, 0, 0:w],
                            start=st, stop=False,
                        )
                        nc.tensor.matmul(
                            pp[:, j, 0:w], w1[p][:, mg], xall[:, 1, 0:w],
                            start=False, stop=True,
                        )
                    qt = qk.tile([128, 2, wmax], F16, tag=f"qk{br}{g}",
                                 name=f"qk{s}{br}{g}")
                    if g == 0:
                        nc.scalar.copy(out=qt[:, :, 0:w], in_=pp[:, :, 0:w])
                    else:
                        nc.vector.tensor_copy(
                            out=qt[:, :, 0:w], in_=pp[:, :, 0:w])
                    if s in cons_off and w < wmax:
                        nc.vector.memset(qt[:, :, w:wmax], 0.0)
                    qkt[(br, g)] = qt
            S_qkt[s] = qkt
            return qkt

        def tail_front(il, w, E, npn_ap, npp_ap):
            """DVE: 8 row-sums (4x), signed Sc correction, recip, 8 prescales.
            Returns the P tile of scaled maps."""
            Sall = small.tile([128, 8], F32, tag="Sall")
            sdum = work.tile([128, wpad], F16, tag="sdum")
            for h8 in range(8):
                nc.vector.tensor_scalar(
                    out=sdum[0:il, 0:w], in0=E[0:il, h8, 0:w],
                    scalar1=1.0, scalar2=0.0, op0=AL.mult, op1=AL.add,
                    accum_out=Sall[0:il, h8 : h8 + 1],
                )
            rsig = small.tile([128, 8], F32, tag="rsig")
            nc.vector.tensor_scalar(
                out=rsig[0:il, 0:4], in0=Sall[0:il, 0:4],
                scalar1=1.0, scalar2=npn_ap, op0=AL.mult, op1=AL.add,
            )
            nc.vector.tensor_scalar(
                out=rsig[0:il, 4:8], in0=Sall[0:il, 4:8],
                scalar1=-1.0, scalar2=npp_ap, op0=AL.mult, op1=AL.add,
            )
            r = small.tile([128, 8], F32, tag="r")
            nc.vector.reciprocal(out=r[0:il], in_=rsig[0:il])
            P = epool.tile([128, 8, wmax], F16, tag="P")
            for h8 in range(8):
                nc.vector.tensor_scalar(
                    out=P[0:il, h8, 0:w], in0=E[0:il, h8, 0:w],
                    scalar1=r[0:il, h8 : h8 + 1], scalar2=None, op0=AL.mult,
                )
            return P

        def tail_combine(il, w, P, A_ap, dve_tree=False):
            """Sum 8 scaled maps + A, then square (ACT). Normally PE
            identity-matmuls accumulate in PSUM f32; alternate end-game
            chunks use a DVE add-tree instead so PE and DVE drain in
            parallel once no more score matmuls remain."""
            msq = work.tile([128, wpad], F16, tag="msq")
            mAp = pscp.tile([128, 2, 512], F32, tag="psc")
            mA = mAp[0:il, 0, 0:w]
            for h8 in range(8):
                nc.tensor.matmul(
                    mA, eye[0:il, 0:il], P[0:il, h8, 0:w],
                    start=(h8 == 0), stop=False,
                )
            nc.tensor.matmul(mA, eye[0:il, 0:il], A_ap, start=False, stop=True)
            nc.scalar.activation(out=msq[0:il, 0:w], in_=mA, func=AF.Square)
            return msq

        def tail_final(il, w, msq, G_ap, res_targets):
            """DVE: fused mask-multiply + row-reduce into res columns."""
            eng = nc.vector
            sdum2 = work.tile([128, wpad], F16, tag="sdum2")
            kind, tgt = res_targets
            if kind == "direct":
                eng.scalar_tensor_tensor(
                    out=sdum2[0:il, 0:w], in0=msq[0:il, 0:w],
                    scalar=1.0, in1=G_ap, op0=AL.mult, op1=AL.mult,
                    accum_out=tgt,
                )
            else:
                eng.scalar_tensor_tensor(
                    out=sdum2[0:il, 0:w], in0=msq[0:il, 0:w],
                    scalar=1.0, in1=G_ap, op0=AL.mult, op1=AL.mult,
                    accum_out=consc[0:il],
                )
                for p0, p1, col in tgt:
                    nc.vector.tensor_copy(
                        out=res[p0:p1, col : col + 1], in_=consc[p0:p1]
                    )

        # software pipelining: chunk k's combine+final emit after chunk k+1's
        # scores+exp, so late-dependency PE/ACT/DVE ops never head-of-line
        # block the next chunk's early work.
        pipe = {"depth": 5}
        pend_q = []

        def flush_pending(all_=False):
            while pend_q and (all_ or len(pend_q) >= pipe["depth"]):
                pend_q.pop(0)()

        # ---- per-slot processing (chunks 0..1 only; rest consolidated) ----
        def do_slot(s, qkt=None):
            n = slot_n[s]
            w = n
            xall, amtile, npf = preloaded[s]
            if qkt is None:
                qkt = project(s, n, xall)
            ncha = (n + 127) // 128
            nch = min(2, ncha) if s in cons_off else ncha
            for ic in range(nch):
                i0 = 128 * ic
                il = min(128, n - i0)
                E = epool.tile([128, 8, wmax], F16, tag="E")
                for half in range(2):
                    for pair in range(2):
                        psc = pscp.tile([128, 2, 512], F32, tag="psc")
                        for bank in range(2):
                            m = 4 * half + 2 * pair + bank
                            br, h = m // 4, m % 4
                            g, sub = h // 2, h % 2
                            rows = slice(64 * sub, 64 * sub + 64)
                            nc.tensor.matmul(
                                psc[0:il, bank, 0:w],
                                qkt[(br, g)][rows, 0, i0 : i0 + il],
                                qkt[(br, g)][rows, 1, 0:w],
                                start=True, stop=True,
                            )
                        nc.scalar.activation(
                            out=E[0:il, 4 * half + 2 * pair : 4 * half + 2 * pair + 2, 0:w],
                            in_=psc[0:il, :, 0:w],
                            func=AF.Exp,
                        )
                flush_pending()
                P = tail_front(
                    il, w, E,
                    npn_ap=npf[0:il, 0:1],
                    npp_ap=npf[0:il, 1:2],
                )
                A_ap = amtile[0:il, ic, 0:w]
                G_ap = amtile[0:il, 3 + ic, 0:w]
                tgt = ("direct", res[0:il, s * 3 + ic : s * 3 + ic + 1])

                def mk(il=il, w=w, P=P, A_ap=A_ap, G_ap=G_ap, tgt=tgt,
                       dve_tree=False):
                    msq = tail_combine(il, w, P, A_ap, dve_tree=dve_tree)
                    tail_final(il, w, msq, G_ap, tgt)

                pend_q.append(mk)

        def do_cons():
            amc = xpool.tile([128, 7, wpad], F16, tag="am", name="amcons")
            nc.sync.dma_start(
                out=amc, in_=amap_d[BPC].rearrange("(c p) w -> p c w", p=128))
            npfc = xpool.tile([128, 2], F32, tag="npf", name="npfcons")
            nc.vector.tensor_copy(out=npfc, in_=amc[:, 6, wmax : wmax + 2])

            E = epool.tile([128, 8, wmax], F16, tag="E")
            for half in range(2):
                for pair in range(2):
                    psc = pscp.tile([128, 2, 512], F32, tag="psc")
                    for bank in range(2):
                        m = 4 * half + 2 * pair + bank
                        br, h = m // 4, m % 4
                        g, sub = h // 2, h % 2
                        rows = slice(64 * sub, 64 * sub + 64)
                        for s in bigs:
                            n = slot_n[s]
                            cc = n - 256
                            p0 = cons_off[s]
                            nc.tensor.matmul(
                                psc[p0 : p0 + cc, bank, 0:wmax],
                                S_qkt[s][(br, g)][rows, 0, 256:n],
                                S_qkt[s][(br, g)][rows, 1, 0:wmax],
                                start=True, stop=True,
                            )
                    nc.scalar.activation(
                        out=E[0:ptot, 4 * half + 2 * pair : 4 * half + 2 * pair + 2, 0:wmax],
                        in_=psc[0:ptot, :, 0:wmax],
                        func=AF.Exp,
                    )
            flush_pending()
            P = tail_front(
                ptot, wmax, E,
                npn_ap=npfc[0:ptot, 0:1],
                npp_ap=npfc[0:ptot, 1:2],
            )
            A_ap = amc[0:ptot, 0, 0:wmax]
            G_ap = amc[0:ptot, 3, 0:wmax]
            tgt = (
                "cons",
                [
                    (cons_off[s], cons_off[s] + slot_n[s] - 256, s * 3 + 2)
                    for s in bigs
                ],
            )

            def mk(P=P, A_ap=A_ap, G_ap=G_ap, tgt=tgt, dve_tree=False):
                msq = tail_combine(ptot, wmax, P, A_ap, dve_tree=dve_tree)
                tail_final(ptot, wmax, msq, G_ap, tgt)

            pend_q.append(mk)

        order_s = [s for s in range(BPC - 1, -1, -1) if s in cons_off]
        order_s += [s for s in range(BPC - 1, -1, -1) if s not in cons_off]
        preloaded = {}
        for s in order_s:
            preloaded[s] = load_slot(s, slot_n[s], sum(slot_n[:s]))
        done = 0
        for s in order_s:
            do_slot(s)
            done += 1
            if done == BPC - 1 and bigs:
                do_cons()
        flush_pending(all_=True)

        # ---- final: sum over partitions, fold 3 cols/slot ----
        pfin_t = pscp.tile([128, 2, 512], F32, tag="psc")
        pfin = pfin_t[0:1, 0, 0 : BPC * 3]
        nc.tensor.matmul(pfin, ones128, res, start=True, stop=True)
        fin_sb = singles.tile([1, BPC, 3], F32, tag="fin_sb")
        nc.vector.tensor_copy(
            out=fin_sb,
            in_=pfin_t[0:1, 0, 0 : BPC * 3].rearrange("p (s c) -> p s c", c=3),
        )
        lt = singles.tile([1, BPC], F32, tag="lt")
        nc.vector.tensor_add(lt, fin_sb[:, :, 0], fin_sb[:, :, 1])
        nc.vector.tensor_add(lt, lt, fin_sb[:, :, 2])
        nc.sync.dma_start(out=loss_d, in_=lt)

    nc.compile()
    return nc


def _prep(inputs):
    me = np.asarray(inputs["molecule_embedding"], np.float32)
    src_mask = np.asarray(inputs["src_mask"]).astype(bool)
    tgt_mask = np.asarray(inputs["tgt_mask"]).astype(bool)
    src_bond = np.asarray(inputs["src_bond"]).astype(np.int64)
    tgt_bond = np.asarray(inputs["tgt_bond"]).astype(np.int64)

    def f64(k):
        return np.asarray(inputs[k], np.float64)

    # compose (pointwise conv -> in_proj) into one weight; q side gets hd^-.5
    # and keeps its bias (as rank-1 matmul rows); k side drops its bias
    # (softmax is invariant to per-row score shifts).
    wall = np.zeros((4, DIM, DIM), NP_EDT)
    wqb = np.zeros((1, 4, 128), NP_EDT)
    for p, (pre, qk_) in enumerate(
        (("inc", "q"), ("inc", "k"), ("dec", "q"), ("dec", "k"))
    ):
        w2, b2 = f64(f"{pre}_w{qk_}"), f64(f"{pre}_b{qk_}")
        w1, b1 = f64(f"{pre}_{qk_}_w"), f64(f"{pre}_{qk_}_b")
        W = w2 @ w1
        bvec = w2 @ b1 + b2
        if qk_ == "q":
            W *= HD ** -0.5
            bvec *= HD ** -0.5
            br = p // 2
            wqb[0, 2 * br + 0, :] = bvec[0:128].astype(NP_EDT)
            wqb[0, 2 * br + 1, :] = bvec[128:256].astype(NP_EDT)
        wall[p] = W.T.astype(NP_EDT)

    kept = [np.nonzero(~src_mask[b])[0] for b in range(B)]
    nk = np.array([len(k) for k in kept])
    order = np.argsort(nk, kind="stable")
    slot_n = []
    for s in range(BPC):
        mx = nk[order[s * NCORES : (s + 1) * NCORES]].max()
        slot_n.append(int(-(-mx // 8) * 8))
    totc = int(sum(slot_n))
    nmax = max(max(slot_n), 64)
    assert nmax <= 504
    wmax = nmax
    wpad = wmax + 8
    bigs, cons_off, ptot = _cons_split(slot_n)

    in_maps = []
    for c in range(NCORES):
        xT = np.zeros((DIM, totc), NP_EDT)
        amap = np.zeros((BPC + 1, 7, 128, wpad), NP_EDT)

        off = 0
        for s in range(BPC):
            n = slot_n[s]
            w = n
            b = int(order[s * NCORES + c])
            kb = kept[b]
            m = len(kb)
            xT[0:DIM, off : off + m] = me[kb, b, :].T.astype(NP_EDT)
            # npad scalar columns: col wmax = -npad, col wmax+1 = +npad
            npad = float(w - m)
            amap[s, 6, :, wmax] = -npad
            amap[s, 6, :, wmax + 1] = npad
            tb = tgt_mask[b, kb].astype(np.float32)

            # A = C_src - C_tgt * (1-t_i)(1-t_j), compact
            remap = np.full(L, -1, np.int64)
            remap[kb] = np.arange(m)
            A = np.zeros((m, m), np.float32)
            rows = np.repeat(np.arange(m), MB)
            cs = remap[src_bond[b, kb, :]].ravel()
            ok = cs >= 0
            np.add.at(A, (rows[ok], cs[ok]), 1.0)
            Ct = np.zeros((m, m), np.float32)
            ct_ = remap[tgt_bond[b, kb, :]].ravel()
            ok = ct_ >= 0
            np.add.at(Ct, (rows[ok], ct_[ok]), 1.0)
            A -= Ct * np.outer(1.0 - tb, 1.0 - tb)
            Af = A.astype(NP_EDT)
            # g = (1 - t_i t_j) on valid rows/cols, 0 elsewhere
            G = (1.0 - np.outer(tb, tb)).astype(NP_EDT)

            ncha = (n + 127) // 128
            nch = min(2, ncha) if s in cons_off else ncha
            for ic in range(nch):
                i0 = ic * 128
                ilr = max(0, min(128, m - i0))
                if ilr > 0:
                    amap[s, ic, 0:ilr, 0:m] = Af[i0 : i0 + ilr, :]
                    amap[s, 3 + ic, 0:ilr, 0:m] = G[i0 : i0 + ilr, :]
            if s in cons_off:
                p0 = cons_off[s]
                npc = float(wmax - m)
                amap[BPC, 6, p0 : p0 + 32, wmax] = -npc
                amap[BPC, 6, p0 : p0 + 32, wmax + 1] = npc
                ilr = max(0, m - 256)
                if ilr > 0:
                    amap[BPC, 0, p0 : p0 + ilr, 0:m] = Af[256 : 256 + ilr, :]
                    amap[BPC, 3, p0 : p0 + ilr, 0:m] = G[256 : 256 + ilr, :]
            off += w

        in_maps.append(
            {
                "xT": xT,
                "wall": wall,
                "wqb": wqb,
                "eye": np.eye(128, dtype=NP_EDT),
                "amap": amap.reshape(BPC + 1, 7 * 128, wpad),
            }
        )
    return in_maps, tuple(slot_n), nmax, totc, order


def kernel(**inputs) -> np.ndarray:
    in_maps, slot_n, nmax, totc, order = _prep(inputs)
    key = (slot_n, nmax, totc, str(EDT))
    if key not in _CACHE:
        _CACHE[key] = _build_program(list(slot_n), nmax, totc)
    nc = _CACHE[key]
    res = bass_utils.run_bass_kernel_spmd(
        nc,
        in_maps,
        core_ids=list(range(NCORES)),
        trace=False,
    )
    global LAST_RESULTS
    LAST_RESULTS = res
    loss = np.zeros(B, np.float32)
    for c in range(NCORES):
        per_core = res.results[c]["loss"].reshape(BPC)
        for s in range(BPC):
            loss[order[s * NCORES + c]] = per_core[s]
    return loss


LAST_RESULTS = None
